# revision 16
# baseline (speedup 1.0000x reference)
"""Trainium2 Bass kernel for CustomMultiheadAttention.

Shapes: query/key_/value [S=2048, B=2, E=1024] f32, H=16 heads, D=64.
Returns (out [S,B,E], attn [B,H,S,S]) like the torch/jax reference.

Sharding: tensor-parallel over heads — each of the 8 NeuronCores computes
2 heads' Q/K/V projections, full attention for its (batch, head) pairs,
its attention-probability output slice, and a partial output projection
(summed on host).

Perf notes (TRN2): plain fp32 matmul streams at 4 cycles/row; float32r
(TF32-like rounded fp32) streams at 1 cycle/row when the moving dim is
>=256, so every large matmul here uses float32r operands. The softmax
itself (exp + normalize, via ScalarE accum_out for the denominator) is
exact fp32, so attention probabilities keep ~1e-4 accuracy.
"""

import numpy as np

S, B, E, H = 2048, 2, 1024, 16
D = E // H  # 64
N_CORES = 8
HC = H // N_CORES  # heads per core = 2
DLOC = HC * D  # 128
T = S * B  # 4096 tokens, b-major (t = b*S + s)

F32 = np.float32

_CACHE = {}


def _build_nc():
    import concourse.bacc as bacc
    import concourse.mybir as mybir
    import concourse.tile as tile
    from concourse.masks import make_identity

    dt = mybir.dt.float32
    dtr = mybir.dt.float32r
    dtb = mybir.dt.bfloat16
    AF = mybir.ActivationFunctionType

    nc = bacc.Bacc(None, target_bir_lowering=False, debug=False)

    xt_q = nc.dram_tensor("xt_q", [E, T], dt, kind="ExternalInput")
    xt_k = nc.dram_tensor("xt_k", [E, T], dt, kind="ExternalInput")
    xt_v = nc.dram_tensor("xt_v", [E, T], dt, kind="ExternalInput")
    wt_q = nc.dram_tensor("wt_q", [E, DLOC], dt, kind="ExternalInput")
    wt_k = nc.dram_tensor("wt_k", [E, DLOC], dt, kind="ExternalInput")
    wt_v = nc.dram_tensor("wt_v", [E, DLOC], dt, kind="ExternalInput")
    wot = nc.dram_tensor("wot", [DLOC, E], dt, kind="ExternalInput")
    b_q = nc.dram_tensor("b_q", [DLOC, 1], dt, kind="ExternalInput")
    b_k = nc.dram_tensor("b_k", [DLOC, 1], dt, kind="ExternalInput")
    b_v = nc.dram_tensor("b_v", [DLOC, 1], dt, kind="ExternalInput")
    bias_m = nc.dram_tensor("bias_m", [S, S], dtb, kind="ExternalInput")

    attn_out = nc.dram_tensor("attn_out", [B, HC, S, S], dtr, kind="ExternalOutput")
    out_part = nc.dram_tensor("out_part", [T, E], dt, kind="ExternalOutput")

    NQG = 4  # q groups of 512 rows
    NKT = S // 128  # 16 k tiles per batch

    with tile.TileContext(nc) as tc:
        with (
            tc.tile_pool(name="const", bufs=1) as const,
            tc.tile_pool(name="wpool", bufs=1) as wpool,
            tc.tile_pool(name="xpool", bufs=2) as xpool,
            tc.tile_pool(name="act", bufs=1) as act,
            tc.tile_pool(name="vtp", bufs=1) as vtp,
            tc.tile_pool(name="biasp", bufs=1) as biasp,
            tc.tile_pool(name="attnp", bufs=8) as attnp,
            tc.tile_pool(name="ptp", bufs=3) as ptp,
            tc.tile_pool(name="denp", bufs=8) as denp,
            tc.tile_pool(name="outp", bufs=1) as outp,
            tc.tile_pool(name="ps_big", bufs=2, space="PSUM") as ps_big,
            tc.tile_pool(name="ps_med", bufs=2, space="PSUM") as ps_med,
            tc.tile_pool(name="ps_ctx", bufs=2, space="PSUM") as ps_ctx,
        ):
            # ---- constants ----
            ident = const.tile([128, 128], dt)
            make_identity(nc, ident[:])
            ident_r = const.tile([128, 128], dtr)
            nc.vector.tensor_copy(ident_r[:], ident[:])
            ident_b = const.tile([128, 128], dtb)
            nc.vector.tensor_copy(ident_b[:], ident[:])

            # ---- weights (cast to f32r on load) ----
            wq_sb = wpool.tile([128, 8, DLOC], dtr)
            wk_sb = wpool.tile([128, 8, DLOC], dtr)
            wv_sb = wpool.tile([128, 8, DLOC], dtr)
            nc.gpsimd.dma_start(wq_sb[:], wt_q[:, :].rearrange("(a p) m -> p a m", p=128))
            nc.gpsimd.dma_start(wk_sb[:], wt_k[:, :].rearrange("(a p) m -> p a m", p=128))
            nc.gpsimd.dma_start(wv_sb[:], wt_v[:, :].rearrange("(a p) m -> p a m", p=128))
            wot_sb = wpool.tile([128, E], dtr)
            nc.gpsimd.dma_start(wot_sb[:], wot[:, :])
            bq_sb = wpool.tile([DLOC, 1], dt)
            bk_sb = wpool.tile([DLOC, 1], dt)
            bv_sb = wpool.tile([DLOC, 1], dt)
            nc.sync.dma_start(bq_sb[:], b_q[:, :])
            nc.sync.dma_start(bk_sb[:], b_k[:, :])
            nc.sync.dma_start(bv_sb[:], b_v[:, :])

            # ---- persistent activations ----
            qt_sb = act.tile([128, T], dtr)  # Q^T  [dloc, token]
            kt_sb = act.tile([128, T], dtr)  # K^T  [dloc, token]
            v_sb = act.tile([128, T // 128, DLOC], dtr)  # V [token, dloc]
            ctxT0 = act.tile([128, S], dtr)  # context^T for batch 0
            ctxT1 = act.tile([128, S], dtr)
            ctxT = [ctxT0, ctxT1]

            # ---- Q, K, V projections (Y^T layout [dloc, token]) ----
            for name, xt, w_sb, b_sb in (
                ("q", xt_q, wq_sb, bq_sb),
                ("k", xt_k, wk_sb, bk_sb),
                ("v", xt_v, wv_sb, bv_sb),
            ):
                for nt in range(T // 512):
                    x_t = xpool.tile([128, 8, 512], dtr, tag="xt")
                    nc.gpsimd.dma_start(
                        x_t[:],
                        xt[:, :].rearrange("(a p) t -> p a t", p=128)[
                            :, :, nt * 512 : (nt + 1) * 512
                        ],
                    )
                    ps = ps_med.tile([128, 512], dt, tag="mm512")
                    for a in range(8):
                        nc.tensor.matmul(
                            ps[:],
                            w_sb[:, a, :],
                            x_t[:, a, :],
                            start=(a == 0),
                            stop=(a == 7),
                        )
                    if name != "v":
                        y_sb = qt_sb if name == "q" else kt_sb
                        nc.scalar.activation(
                            y_sb[:, nt * 512 : (nt + 1) * 512],
                            ps[:],
                            AF.Identity,
                            bias=b_sb[:],
                        )
                    else:
                        vt_t = vtp.tile([128, 512], dtr)
                        nc.scalar.activation(
                            vt_t[:], ps[:], AF.Identity, bias=b_sb[:]
                        )
                        # transpose to natural V [token, dloc]
                        for sub in range(4):
                            tt = nt * 4 + sub
                            tps = ps_med.tile([128, 128], dtr, tag="mm512")
                            nc.tensor.transpose(
                                tps[:],
                                vt_t[:, sub * 128 : (sub + 1) * 128],
                                ident_r[:],
                            )
                            nc.vector.tensor_copy(v_sb[:, tt, :], tps[:])

            # ---- attention ----
            # Software-pipelined: emit unit u+1's scores/exp/normalize before
            # unit u's transpose+context loop so PE always has independent
            # work while ScalarE runs exp (keeps the HAM clock-gate open).
            flip = 0
            bias_tiles = {}
            units = [
                (qg, b, h)
                for qg in range(NQG)
                for b in range(B)
                for h in range(HC)
            ]

            def emit_scores(qg, b, h):
                d0 = h * D
                p_ts = []
                for qj in range(4):
                    qb = qg * 4 + qj
                    if b == 0 and h == 0:
                        bias_t = biasp.tile([128, S], dtb, tag=f"bias{qj}")
                        nc.sync.dma_start(
                            bias_t[:], bias_m[qb * 128 : (qb + 1) * 128, :]
                        )
                        bias_tiles[qb] = bias_t
                    else:
                        bias_t = bias_tiles[qb]
                    p_t = attnp.tile([128, S], dtr, name="p_t", tag="p_t")
                    dens = []
                    for kh in range(2):
                        sc = ps_big.tile([128, 1024], dt, tag="scores")
                        for kn2 in range(2):
                            kn = kh * 2 + kn2
                            ksl = slice(kn2 * 512, (kn2 + 1) * 512)
                            nc.tensor.matmul(
                                sc[:, ksl],
                                ident_b[:],
                                bias_t[:, kn * 512 : (kn + 1) * 512],
                                start=True,
                                stop=False,
                            )
                            nc.tensor.matmul(
                                sc[:, ksl],
                                qt_sb[
                                    d0 : d0 + D,
                                    b * S + qb * 128 : b * S + (qb + 1) * 128,
                                ],
                                kt_sb[
                                    d0 : d0 + D,
                                    b * S + kn * 512 : b * S + (kn + 1) * 512,
                                ],
                                start=False,
                                stop=True,
                            )
                        den_h = denp.tile([128, 1], dt, tag=f"den{kh}")
                        nc.scalar.activation(
                            p_t[:, kh * 1024 : (kh + 1) * 1024],
                            sc[:],
                            AF.Exp,
                            accum_out=den_h[:],
                        )
                        dens.append(den_h)
                    rec = denp.tile([128, 1], dt, tag="rec")
                    nc.vector.tensor_add(rec[:], dens[0][:], dens[1][:])
                    nc.vector.reciprocal(rec[:], rec[:])
                    nc.vector.tensor_scalar_mul(p_t[:], p_t[:], rec[:])
                    nc.sync.dma_start(
                        attn_out[b, h, qb * 128 : (qb + 1) * 128, :], p_t[:]
                    )
                    p_ts.append(p_t)
                return p_ts

            def emit_context(qg, b, h, p_ts):
                nonlocal flip
                d0 = h * D
                ctx = ps_ctx.tile([64, 512], dt, tag="ctx")
                for kt in range(NKT):
                    pt_ps = ps_med.tile([128, 512], dtr, tag="mm512")
                    for qj in range(4):
                        nc.tensor.transpose(
                            pt_ps[:, qj * 128 : (qj + 1) * 128],
                            p_ts[qj][:, kt * 128 : (kt + 1) * 128],
                            ident_r[:],
                        )
                    pt_sb = ptp.tile([128, 512], dtr, name="pt_sb", tag="pt_sb")
                    if flip % 2 == 0:
                        nc.scalar.copy(pt_sb[:], pt_ps[:])
                    else:
                        nc.vector.tensor_copy(pt_sb[:], pt_ps[:])
                    flip += 1
                    nc.tensor.matmul(
                        ctx[:],
                        v_sb[:, b * NKT + kt, d0 : d0 + D],
                        pt_sb[:],
                        start=(kt == 0),
                        stop=(kt == NKT - 1),
                    )
                nc.vector.tensor_copy(
                    ctxT[b][d0 : d0 + D, qg * 512 : (qg + 1) * 512], ctx[:]
                )

            pending = emit_scores(*units[0])
            for u, unit in enumerate(units):
                nxt = emit_scores(*units[u + 1]) if u + 1 < len(units) else None
                emit_context(*unit, pending)
                pending = nxt

            # ---- output projection (partial; host reduces across cores) ----
            for b in range(B):
                for tt in range(S // 128):
                    o_row = outp.tile([128, E], dt)
                    for en in range(E // 512):
                        ps = ps_med.tile([128, 512], dt, tag="mm512")
                        nc.tensor.matmul(
                            ps[:],
                            ctxT[b][:, tt * 128 : (tt + 1) * 128],
                            wot_sb[:, en * 512 : (en + 1) * 512],
                            start=True,
                            stop=True,
                        )
                        nc.vector.tensor_copy(
                            o_row[:, en * 512 : (en + 1) * 512], ps[:]
                        )
                    t0 = b * S + tt * 128
                    nc.sync.dma_start(out_part[t0 : t0 + 128, :], o_row[:])

    nc.compile()
    return nc


def _get_nc():
    if "nc" not in _CACHE:
        _CACHE["nc"] = _build_nc()
    return _CACHE["nc"]


def kernel(query, key_, value, Wq, bq, Wk, bk, Wv, bv, Wo, bo, bias_matrix):
    import ml_dtypes
    from concourse.bass_utils import run_bass_kernel_spmd

    query = np.asarray(query, F32)
    key_ = np.asarray(key_, F32)
    value = np.asarray(value, F32)
    Wq = np.asarray(Wq, F32)
    bq = np.asarray(bq, F32)
    Wk = np.asarray(Wk, F32)
    bk = np.asarray(bk, F32)
    Wv = np.asarray(Wv, F32)
    bv = np.asarray(bv, F32)
    Wo = np.asarray(Wo, F32)
    bo = np.asarray(bo, F32)
    bias_matrix = np.asarray(bias_matrix, F32)

    # [S,B,E] -> X^T [E, T] with tokens b-major (t = b*S + s)
    xt_q = np.ascontiguousarray(query.transpose(2, 1, 0).reshape(E, T))
    xt_k = np.ascontiguousarray(key_.transpose(2, 1, 0).reshape(E, T))
    xt_v = np.ascontiguousarray(value.transpose(2, 1, 0).reshape(E, T))
    bias_m = np.ascontiguousarray(bias_matrix.astype(ml_dtypes.bfloat16))

    scale = F32(D ** -0.5)
    WqT = np.ascontiguousarray(Wq.T) * scale  # [E_in, E_out], scaled
    WkT = np.ascontiguousarray(Wk.T)
    WvT = np.ascontiguousarray(Wv.T)
    WoT = np.ascontiguousarray(Wo.T)  # [E_in(=head dims), E_out]

    in_maps = []
    for c in range(N_CORES):
        rows = slice(c * DLOC, (c + 1) * DLOC)
        in_maps.append(
            {
                "xt_q": xt_q,
                "xt_k": xt_k,
                "xt_v": xt_v,
                "wt_q": np.ascontiguousarray(WqT[:, rows]),
                "wt_k": np.ascontiguousarray(WkT[:, rows]),
                "wt_v": np.ascontiguousarray(WvT[:, rows]),
                "wot": np.ascontiguousarray(WoT[rows, :]),
                "b_q": np.ascontiguousarray((bq[rows] * scale).reshape(DLOC, 1)),
                "b_k": np.ascontiguousarray(bk[rows].reshape(DLOC, 1)),
                "b_v": np.ascontiguousarray(bv[rows].reshape(DLOC, 1)),
                "bias_m": bias_m,
            }
        )

    global last_in_maps
    last_in_maps = in_maps
    nc = _get_nc()
    res = run_bass_kernel_spmd(nc, in_maps, core_ids=list(range(N_CORES)))

    attn = np.empty((B, H, S, S), F32)
    out_t = np.zeros((T, E), F32)
    for c in range(N_CORES):
        attn[:, c * HC : (c + 1) * HC] = res.results[c]["attn_out"]
        out_t += res.results[c]["out_part"]
    out = out_t.reshape(B, S, E) + bo  # tokens are b-major
    out = np.ascontiguousarray(out.transpose(1, 0, 2))  # [S, B, E]
    return out, attn
